# revision 1
# baseline (speedup 1.0000x reference)
"""Trainium2 Bass kernel for the correlation-softargmax flow module.

Math (per batch b, query pixel q=(y,x)):
  c1 = l2norm_C(feature1), warp = l2norm_C(feature2)
  s[l,q] = <3x3 patch of warp at l, 3x3 patch of c1 at q>    (D = 32*9 = 288)
  p = softmax_l(10*s);  flow = (E_p[ix_l] - x, E_p[iy_l] - y)

Key factorization: the 3x3 tap sum is separable over (dy, dx) and each tap
shifts BOTH l and q:  s = sum_dx A[(ly,lx+dx),(qy,qx+dx)]  where
A = sum_{c,dy} warp[c,ly+dy,lx]*c1[c,qy+dy,qx]  (y-taps folded into one K=96
matmul).  The x-tap sum is applied AFTER the exp via
exp(a+b+c) = exp(a)*exp(b)*exp(c): E = exp(10*A - 10) and the dx=+-1 factors
are diagonal (partition+-1, free+-1) shifted copies of E, produced by DMA with
boundary cells pre-filled with exp(-10) (the "A=0 out-of-image" factor).  Two
bf16 DVE multiplies then give p = E*Em1*Ep1 = exp(10*s - 30), halving the PE
matmul work per tile (one K=96 score matmul + one stats matmul instead of
three score matmuls + stats).

Because softmax normalizes, only Z = sum_l p, Sy = sum_l p*iy, Sx = sum_l p*ix
are needed per q (flash-attention style, no [L,L] materialization; the fixed
-30 shift keeps everything in range since |10*s| <= 90).

Sharding: 8 cores = 4 batches x 2 query-row halves. Each core holds the full
K-side image (softmax runs over all 4096 l) and 2048 queries.
"""

import sys

import numpy as np

sys.path.insert(0, "/opt/trn_rl_repo")

import concourse.bass as bass  # noqa: E402
import concourse.mybir as mybir  # noqa: E402
import concourse.tile as tile  # noqa: E402
from concourse import bacc, bass_utils  # noqa: E402

F32 = mybir.dt.float32
F32R = mybir.dt.float32r
F16 = mybir.dt.float16
BF16 = mybir.dt.bfloat16

B, C, H, W = 4, 32, 64, 64
L = H * W              # 4096 match locations
NQ = L // 2            # queries per core
QROWS = H // 2         # query rows per core
NQT = NQ // 128        # epilogue bounce columns
N_CORES = 8
SCALE = 10.0
SHIFT1 = -10.0         # per-factor shift; 3 factors -> exp(10*s - 30)
EM_BORDER = float(np.exp(-10.0))   # exp(10*0 - 10): out-of-image tap factor
EPS = 1e-12

_NC_CACHE = {}
_LAST_RES = None


def _build_nc():
    nc = bacc.Bacc(None, target_bir_lowering=False)

    f1h = nc.dram_tensor("f1h", [C, QROWS + 2, W], F32, kind="ExternalInput")
    f2 = nc.dram_tensor("f2", [C, H, W], F32, kind="ExternalInput")
    w3 = nc.dram_tensor("w3", [128, 96], F32, kind="ExternalInput")
    yqt = nc.dram_tensor("yqt", [128, NQT], F32, kind="ExternalInput")
    xqt = nc.dram_tensor("xqt", [128, NQT], F32, kind="ExternalInput")
    outp = nc.dram_tensor("outp", [2, NQ], F32, kind="ExternalOutput")

    n1 = (QROWS + 2) * W   # 2176 pixels in the f1 halo slab
    n_lt = L // 128        # 32 l-tiles
    n_qt = NQ // 512       # 4 q-tiles
    DELAY = 3              # stats matmul trails its p by this many rounds

    with tile.TileContext(nc) as tc:
        with tc.tile_pool(name="big", bufs=1) as big, \
             tc.tile_pool(name="work", bufs=1) as work, \
             tc.tile_pool(name="small", bufs=1) as small, \
             tc.tile_pool(name="pp", bufs=1) as pp, \
             tc.tile_pool(name="epi", bufs=1) as epi, \
             tc.tile_pool(name="nps", bufs=2, space="PSUM") as nps, \
             tc.tile_pool(name="sps", bufs=4, space="PSUM") as sps, \
             tc.tile_pool(name="stps", bufs=2, space="PSUM") as stps:

            # ---- load inputs (images packed 4 channel-blocks deep so the
            # squares run 128 partitions wide) ----
            XS2 = L // 4           # 1024 pixels per f2 block
            XS1 = n1 // 4          # 544 pixels per f1 block
            raw2 = big.tile([128, XS2], F32, tag="raw2")
            f2f = f2.rearrange("c h w -> c (h w)")
            raw1 = big.tile([128, XS1], F32, tag="raw1")
            f1f = f1h.rearrange("c h w -> c (h w)")
            for j in range(4):
                nc.sync.dma_start(out=raw2[32 * j:32 * j + 32, :],
                                  in_=f2f[:, XS2 * j:XS2 * (j + 1)])
                nc.gpsimd.dma_start(out=raw1[32 * j:32 * j + 32, :],
                                    in_=f1f[:, XS1 * j:XS1 * (j + 1)])
            w3f = small.tile([128, 96], F32, tag="w3f")
            nc.scalar.dma_start(out=w3f, in_=w3[:, :])
            xqs = small.tile([128, NQT], F32, tag="xqs")
            nc.scalar.dma_start(out=xqs, in_=xqt[:, :])
            yqs = small.tile([128, NQT], F32, tag="yqs")
            nc.scalar.dma_start(out=yqs, in_=yqt[:, :])

            onesf = small.tile([128, 1], F32, tag="onesf")
            nc.vector.memset(onesf, 1.0)
            ones128 = small.tile([128, 1], F32R, tag="ones128")
            nc.scalar.copy(ones128, onesf)
            shiftc = small.tile([128, 1], F32, tag="shiftc")
            nc.vector.memset(shiftc, SHIFT1)
            eps2c = small.tile([128, 1], F32, tag="eps2c")
            nc.vector.memset(eps2c, EPS * EPS)

            # E/Em1/Ep1 live in an x-padded [128, .., 66] layout: col 0 and 65
            # of each q-row hold exp(-10) (the out-of-image tap factor).  The
            # diagonal shift then becomes ONE contiguous flat copy per
            # partition run: Em1[p, 1:] = E[p-1, :-1] — E's border columns
            # flow into exactly the right border slots of Em1, including
            # across q-row AND tile boundaries, so OCT tiles are shifted by a
            # single pair of DMAs (DMA cost is latency-dominated: ~0.6us
            # fixed per dma_start).  Persistent ring tiles; borders are
            # prefilled once and never overwritten (DMAs/exp write interiors
            # only).
            WP = W + 2
            OCT = 4        # l-tiles per shift/mul batch
            ORR = 8 * OCT  # q-rows per batch tile
            NE = 3         # E ring depth (batch granularity)
            NS = 3         # shift ring depth
            NP = 8         # p ring depth
            et = []
            em1t = []
            ep1t = []
            pt = []
            for i in range(NE):
                e = pp.tile([128, ORR, WP], BF16, tag=f"E_{i}", name=f"E_{i}")
                # only the x-border columns need the exp(-10) fill (interiors
                # are fully rewritten by exp each use)
                nc.gpsimd.memset(e[:, :, 0:1], EM_BORDER)
                nc.gpsimd.memset(e[:, :, W + 1:W + 2], EM_BORDER)
                et.append(e)
            for i in range(NS):
                em = pp.tile([128, ORR, WP], BF16, tag=f"em1_{i}",
                             name=f"em1_{i}")
                nc.gpsimd.memset(em, EM_BORDER)
                em1t.append(em)
                ep = pp.tile([128, ORR, WP], BF16, tag=f"ep1_{i}",
                             name=f"ep1_{i}")
                nc.gpsimd.memset(ep, EM_BORDER)
                ep1t.append(ep)
            for i in range(NP):
                # p/tmp are fully written every use; no prefill needed
                p_sb = pp.tile([128, ORR, WP], BF16, tag=f"p_{i}",
                               name=f"p_{i}")
                pt.append(p_sb)
            tmp = pp.tile([128, ORR, WP], BF16, tag="tmp", name="tmp")

            # ---- l2 normalization over C (C sits on partitions, so the
            # per-pixel sum of squares comes from a ones-vector matmul; the
            # 1/norm row is broadcast back across partitions with a K=1
            # ones-matmul and the scaling multiply reads it from PSUM).
            # Images are padded in y ONLY (x taps are handled post-exp). ----
            sq2 = big.tile([128, XS2], F32R, tag="sq2")
            nc.vector.tensor_mul(sq2, raw2, raw2)
            sq1 = big.tile([128, XS1], F32R, tag="sq1")
            nc.vector.tensor_mul(sq1, raw1, raw1)

            # constants not needed until the scale/stats phases
            w3r = small.tile([128, 96], BF16, tag="w3r")
            nc.scalar.copy(w3r, w3f)
            onesbf = small.tile([1, C], F32, tag="onesbf")
            nc.vector.memset(onesbf, 1.0)
            onesb = small.tile([1, C], F32R, tag="onesb")
            nc.scalar.copy(onesb, onesbf)

            nrow = work.tile([1, L + n1], F32, tag="nrow")
            rrow = work.tile([1, L + n1], F32R, tag="rrow")

            # scaled, y-padded fp16 images (written through flat gap-free
            # views, so scale chunks need no image-row alignment)
            pad2 = big.tile([C, H + 2, W], F16, tag="pad2")
            nc.vector.memset(pad2[:, 0:1, :], 0.0)
            nc.vector.memset(pad2[:, H + 1:H + 2, :], 0.0)
            pad1 = big.tile([C, QROWS + 2, W], F16, tag="pad1")
            pad2f = pad2.rearrange("c h w -> c (h w)")
            pad1f = pad1.rearrange("c h w -> c (h w)")

            # per image: ss -> norms (sqrt folded into PSUM->SBUF copy) ->
            # 128-wide reciprocal -> broadcast + scale.  Image 2 first, so
            # its scale chain (vector) overlaps image 1's sqrt chain (Act).
            def norm_img(sq, XS, base, npix, rawp, padf, pos0, nTw):
                for j in range(4):
                    for (o, n) in ((0, 512), (512, XS - 512)):
                        ssp = nps.tile([1, 512], F32, tag="nps", name="ssp")
                        nc.tensor.matmul(
                            ssp[:, :n], ones128[32 * j:32 * j + 32, :],
                            sq[32 * j:32 * j + 32, o:o + n],
                            start=True, stop=True, tile_position=(32 * j, 0))
                        pos = base + XS * j + o
                        # norm = sqrt(ss + eps^2), folded into the copy
                        nc.scalar.activation(
                            nrow[:, pos:pos + n], ssp[:, :n],
                            mybir.ActivationFunctionType.Sqrt,
                            bias=eps2c[0:1, :])
                nT = work.tile([128, npix // 128], F32, tag=nTw, name=nTw)
                nc.sync.dma_start(
                    out=nT, in_=nrow[:, base:base + npix].rearrange(
                        "a (p c) -> a p c", p=128))
                rT = work.tile([128, npix // 128], F32R, tag=nTw + "r",
                               name=nTw + "r")
                with nc.allow_low_precision(reason="f32r 1/norm, 12 bits"):
                    nc.vector.reciprocal(rT, nT)
                nc.sync.dma_start(
                    out=rrow[:, base:base + npix].rearrange(
                        "a (p c) -> a p c", p=128), in_=rT)
                for j in range(4):
                    for (o, n) in ((0, 512), (512, XS - 512)):
                        rb = nps.tile([C, 512], F32, tag="nps", name="rb")
                        g = XS * j + o
                        nc.tensor.matmul(rb[:, :n], onesb,
                                         rrow[:, base + g:base + g + n],
                                         start=True, stop=True)
                        nc.vector.tensor_mul(  # gpsimd cannot read PSUM
                            padf[:, pos0 + g:pos0 + g + n],
                            rawp[32 * j:32 * j + 32, o:o + n],
                            rb[:, :n],
                        )

            norm_img(sq2, XS2, 0, L, raw2, pad2f, W, "nT2")
            norm_img(sq1, XS1, L, n1, raw1, pad1f, 0, "nT1")

            # ---- d-major y-tap patch tensors: 3 taps of 32 channels (96
            # partitions); each tap is one strided fp16 DMA copy ----
            kc = big.tile([96, H, W], F16, tag="kc")
            qc = big.tile([96, QROWS, W], F16, tag="qc")
            dma_engs = [nc.sync, nc.scalar, nc.sync]
            for j in range(3):
                dma_engs[j].dma_start(out=kc[32 * j:32 * j + 32, :, :],
                                      in_=pad2[:, j:j + H, :])
                dma_engs[j].dma_start(out=qc[32 * j:32 * j + 32, :, :],
                                      in_=pad1[:, j:j + QROWS, :])

            # ---- main loop: A-matmul -> E=exp -> diag shifts -> p -> stats,
            # flash-attention style over l ----
            n_rounds = n_qt * n_lt
            DELAY = 6 * OCT    # stats matmul trails its p by six batches
            stats_t = [None] * n_qt

            def stats_mm(r):
                qt, lt = divmod(r, n_lt)
                p_oct = pt[(r // OCT) % NP]
                i = r % OCT
                nc.tensor.matmul(stats_t[qt], w3r[:, 3 * lt:3 * lt + 3],
                                 p_oct[:, 8 * i:8 * i + 8, 1:W + 1],
                                 start=(lt == 0), stop=(lt == n_lt - 1))
                if lt == n_lt - 1:
                    nc.scalar.copy(stats_sb[:, 512 * qt:512 * (qt + 1)],
                                   stats_t[qt])

            stats_sb = epi.tile([3, NQ], F32, tag="stats_sb", bufs=1)

            for r in range(n_rounds):
                qt, lt = divmod(r, n_lt)
                if lt == 0:
                    stats_t[qt] = stps.tile([3, 512], F32, tag="stats",
                                            name="stats")
                psA = sps.tile([128, 512], F32, tag="A", name="psA")
                nc.tensor.matmul(psA, kc[:, 2 * lt:2 * lt + 2, :],
                                 qc[:, 8 * qt:8 * qt + 8, :],
                                 start=True, stop=True)
                if r >= DELAY:
                    stats_mm(r - DELAY)
                e_sb = et[(r // OCT) % NE]
                i = r % OCT
                nc.scalar.activation(e_sb[:, 8 * i:8 * i + 8, 1:W + 1], psA,
                                     mybir.ActivationFunctionType.Exp,
                                     bias=shiftc, scale=SCALE)
                if i == OCT - 1:
                    o = r // OCT
                    em1 = em1t[o % NS]
                    ep1 = ep1t[o % NS]
                    nf = ORR * WP
                    ef = e_sb.rearrange("p a b -> p (a b)")
                    emf = em1.rearrange("p a b -> p (a b)")
                    epf = ep1.rearrange("p a b -> p (a b)")
                    # diagonal shifts: +1 in both lx (partition) and qx
                    # (free), ONE contiguous DMA per shift on the two HWDGE
                    # queues; the run crosses the lx=0/63 border at one
                    # partition (em1 p=64 / ep1 p=63), repaired by a small
                    # gpsimd memset back to the exp(-10) border value
                    nc.sync.dma_start(out=emf[1:64, 1:nf],
                                      in_=ef[0:63, 0:nf - 1])
                    nc.sync.dma_start(out=emf[65:128, 1:nf],
                                      in_=ef[64:127, 0:nf - 1])
                    nc.gpsimd.dma_start(out=epf[0:63, 0:nf - 1],
                                        in_=ef[1:64, 1:nf])
                    nc.gpsimd.dma_start(out=epf[64:127, 0:nf - 1],
                                        in_=ef[65:128, 1:nf])
                    p_oct = pt[o % NP]
                    nc.vector.tensor_mul(tmp, em1, ep1)
                    nc.vector.tensor_mul(p_oct, tmp, e_sb)
            for r in range(n_rounds - DELAY, n_rounds):
                stats_mm(r)

            # ---- epilogue: flow = S/Z - coord, bounced to [128, NQT] so the
            # reciprocal runs 128-wide ----
            zT = epi.tile([128, NQT], F32, tag="zT")
            nc.sync.dma_start(out=zT,
                              in_=stats_sb[0:1, :].rearrange("a (p c) -> a p c", p=128))
            syT = epi.tile([128, NQT], F32, tag="syT")
            nc.sync.dma_start(out=syT,
                              in_=stats_sb[1:2, :].rearrange("a (p c) -> a p c", p=128))
            sxT = epi.tile([128, NQT], F32, tag="sxT")
            nc.sync.dma_start(out=sxT,
                              in_=stats_sb[2:3, :].rearrange("a (p c) -> a p c", p=128))
            rz = epi.tile([128, NQT], F32R, tag="rz")
            with nc.allow_low_precision(reason="f32r 1/Z, 12 bits"):
                nc.vector.reciprocal(rz, zT)
            fw = epi.tile([128, NQT], F32, tag="fw")
            nc.vector.tensor_mul(fw, sxT, rz)
            nc.vector.tensor_sub(fw, fw, xqs)
            fh = epi.tile([128, NQT], F32, tag="fh")
            nc.vector.tensor_mul(fh, syT, rz)
            nc.vector.tensor_sub(fh, fh, yqs)
            nc.sync.dma_start(
                out=outp[0:1, :].rearrange("a (p c) -> a p c", p=128), in_=fw)
            nc.sync.dma_start(
                out=outp[1:2, :].rearrange("a (p c) -> a p c", p=128), in_=fh)

    nc.finalize()
    return nc


def _host_consts():
    p = np.arange(128)
    w3 = np.zeros((128, 96), np.float32)
    for t in range(32):
        w3[:, 3 * t] = 1.0
        w3[:, 3 * t + 1] = 2 * t + p // 64   # global iy of l = 128*lt + p
        w3[:, 3 * t + 2] = p % 64            # global ix
    # epilogue bounce layout: q = p*NQT + c  ->  [p, c]
    q = np.arange(NQ).reshape(128, NQT)      # [128, NQT], q = NQT*p + c
    xq = (q % W).astype(np.float32)
    ly = (q // W).astype(np.float32)
    return w3, xq, ly


def kernel(feature1, feature2):
    feature1 = np.ascontiguousarray(feature1, np.float32)
    feature2 = np.ascontiguousarray(feature2, np.float32)
    w3, xq, ly = _host_consts()

    f1p = np.zeros((B, C, H + 2, W), np.float32)
    f1p[:, :, 1:H + 1, :] = feature1

    in_maps = []
    for core in range(N_CORES):
        b, h = divmod(core, 2)
        in_maps.append({
            "f1h": np.ascontiguousarray(f1p[b, :, h * QROWS:h * QROWS + QROWS + 2, :]),
            "f2": np.ascontiguousarray(feature2[b]),
            "w3": w3,
            "yqt": ly + h * QROWS,
            "xqt": xq,
        })

    if "nc" not in _NC_CACHE:
        _NC_CACHE["nc"] = _build_nc()
    res = bass_utils.run_bass_kernel_spmd(
        _NC_CACHE["nc"], in_maps, core_ids=list(range(N_CORES)))
    global _LAST_RES
    _LAST_RES = res

    out = np.zeros((B, 2, H, W), np.float32)
    for core in range(N_CORES):
        b, h = divmod(core, 2)
        out[b, :, h * QROWS:(h + 1) * QROWS, :] = (
            res.results[core]["outp"].reshape(2, QROWS, W))
    return out



# revision 6
# speedup vs baseline: 1.3288x; 1.3288x over previous
"""Trainium2 Bass kernel for the correlation-softargmax flow module.

Math (per batch b, query pixel q=(y,x)):
  c1 = l2norm_C(feature1), warp = l2norm_C(feature2)
  s[l,q] = <3x3 patch of warp at l, 3x3 patch of c1 at q>    (D = 32*9 = 288)
  p = softmax_l(10*s);  flow = (E_p[ix_l] - x, E_p[iy_l] - y)

Design ("3mm"): the 9 patch taps factor as (3 y-taps) x (3 x-shifts).  The
y-taps fold into the matmul contraction (K = 32ch * 3 = 96).  The x-shift sum
is done PRE-exp by accumulating three matmuls into one PSUM bank.  One Exp
(scale 10, bias -30 keeps everything finite: |10 s| <= 90) yields p directly;
a [128,3] stats matmul accumulates Z, Sy, Sx over l per query (flash-style).

The l axis is tiled over the X-PADDED image geometry: kc is ONE contiguous
[96, 66*64] tensor (plus guard cells) holding the 3 y-taps of the x-padded
normalized f2; the dx-shifted stationary for l-tile t is the flat window
kc[:, 128t+dx : 128t+dx+128] - a single-free-dim AP as the PE requires, with
no per-dx tensor copies.  Out-of-image taps read the zero pad columns; the
~3% of partition slots that land ON a pad column are junk l-positions whose
stats weights are zero, so they never contribute.  33 l-tiles cover all 4224
padded positions.  Stats matmuls are bundled 4-per-group so the PE pays the
(~100 ns) stationary shape-switch penalty twice per 4 rounds instead of
twice per round.

Per round: PE does 4x512 matmul columns, Act one exp - no DVE multiplies,
no shift DMAs, so the PE stays continuously busy at its full 2.4 GHz p-state.

Sharding: 8 cores = 4 batches x 2 query-row halves. Each core holds the full
K-side image (softmax runs over all 4096 l) and 2048 queries.
"""

import sys

import numpy as np

sys.path.insert(0, "/opt/trn_rl_repo")

import concourse.bass as bass  # noqa: E402
import concourse.mybir as mybir  # noqa: E402
import concourse.tile as tile  # noqa: E402
from concourse import bacc, bass_utils  # noqa: E402

F32 = mybir.dt.float32
F32R = mybir.dt.float32r
F16 = mybir.dt.float16
BF16 = mybir.dt.bfloat16

B, C, H, W = 4, 32, 64, 64
L = H * W              # 4096 match locations
NQ = L // 2            # queries per core
QROWS = H // 2         # query rows per core
SROWS = QROWS + 2      # f1 halo slab rows
NQT = NQ // 128        # epilogue bounce columns
N_CORES = 8
SCALE = 10.0
SHIFT = -30.0          # fixed exp shift: p = exp(10*s - 30), |10 s| <= 90
EPS = 1e-12
WP = W + 2             # x-padded row width (66)
LPAD = WP * H          # 4224 padded l positions
N_LT = LPAD // 128     # 33 l-tiles
N_QT = NQ // 512       # 4 q-tiles

_NC_CACHE = {}
_LAST_RES = None


def _build_nc():
    nc = bacc.Bacc(None, target_bir_lowering=False)

    f1h = nc.dram_tensor("f1h", [C, SROWS, W], F32, kind="ExternalInput")
    f2 = nc.dram_tensor("f2", [C, H, W], F32, kind="ExternalInput")
    w3 = nc.dram_tensor("w3", [128, 3 * N_LT], F32, kind="ExternalInput")
    yqt = nc.dram_tensor("yqt", [128, NQT], F32, kind="ExternalInput")
    xqt = nc.dram_tensor("xqt", [128, NQT], F32, kind="ExternalInput")
    outp = nc.dram_tensor("outp", [2, NQ], F32, kind="ExternalOutput")

    n1 = SROWS * W         # 2176 pixels in the f1 halo slab
    NP = 10                # p ring depth
    # f1 norm blocks must be row-aligned for padded-image writes: rows
    # [9, 9, 8, 8] -> pixel counts [576, 576, 512, 512]
    ROWS1 = [9, 9, 8, 8]
    RST1 = [0, 9, 18, 26]
    PX1 = [r * W for r in ROWS1]

    with tile.TileContext(nc) as tc:
        with tc.tile_pool(name="big", bufs=1) as big, \
             tc.tile_pool(name="work", bufs=1) as work, \
             tc.tile_pool(name="small", bufs=1) as small, \
             tc.tile_pool(name="pp", bufs=1) as pp, \
             tc.tile_pool(name="epi", bufs=1) as epi, \
             tc.tile_pool(name="nps", bufs=2, space="PSUM") as nps, \
             tc.tile_pool(name="sps", bufs=4, space="PSUM") as sps, \
             tc.tile_pool(name="stps", bufs=2, space="PSUM") as stps:

            # ---- load inputs (images packed 4 channel-blocks deep so the
            # squares run 128 partitions wide; f1 blocks are row-aligned) ----
            XS2 = L // 4           # 1024 pixels per f2 block
            XS1 = max(PX1)         # 576-wide f1 block staging
            raw2 = big.tile([128, XS2], F32, tag="raw2")
            f2f = f2.rearrange("c h w -> c (h w)")
            raw1 = big.tile([128, XS1], F32, tag="raw1")
            f1f = f1h.rearrange("c h w -> c (h w)")
            for j in range(4):
                nc.sync.dma_start(out=raw2[32 * j:32 * j + 32, :],
                                  in_=f2f[:, XS2 * j:XS2 * (j + 1)])
                s0 = RST1[j] * W
                nc.gpsimd.dma_start(out=raw1[32 * j:32 * j + 32, 0:PX1[j]],
                                    in_=f1f[:, s0:s0 + PX1[j]])
            w3f = small.tile([128, 3 * N_LT], F32, tag="w3f")
            nc.scalar.dma_start(out=w3f, in_=w3[:, :])
            xqs = small.tile([128, NQT], F32, tag="xqs")
            nc.scalar.dma_start(out=xqs, in_=xqt[:, :])
            yqs = small.tile([128, NQT], F32, tag="yqs")
            nc.scalar.dma_start(out=yqs, in_=yqt[:, :])

            onesf = small.tile([128, 1], F32, tag="onesf")
            nc.vector.memset(onesf, 1.0)
            ones128 = small.tile([128, 1], F32R, tag="ones128")
            nc.scalar.copy(ones128, onesf)
            shiftc = small.tile([128, 1], F32, tag="shiftc")
            nc.vector.memset(shiftc, SHIFT)
            eps2c = small.tile([128, 1], F32, tag="eps2c")
            nc.vector.memset(eps2c, EPS * EPS)

            # ---- l2 normalization over C ----
            sq2 = big.tile([128, XS2], F32R, tag="sq2")
            nc.vector.tensor_mul(sq2, raw2, raw2)
            sq1 = big.tile([128, XS1], F32R, tag="sq1")
            nc.vector.tensor_mul(sq1, raw1, raw1)

            w3r = small.tile([128, 3 * N_LT], BF16, tag="w3r")
            nc.scalar.copy(w3r, w3f)
            onesbf = small.tile([1, C], F32, tag="onesbf")
            nc.vector.memset(onesbf, 1.0)
            onesb = small.tile([1, C], F32R, tag="onesb")
            nc.scalar.copy(onesb, onesbf)

            nrow = work.tile([1, L + n1], F32, tag="nrow")
            rrow = work.tile([1, L + n1], F32R, tag="rrow")

            # x-AND-y padded fp16 images; interiors written by the norm
            # scale multiplies through row-aligned strided views
            pad2 = big.tile([C, H + 2, WP], F16, tag="pad2")
            nc.vector.memset(pad2[:, 0:1, :], 0.0)
            nc.vector.memset(pad2[:, H + 1:H + 2, :], 0.0)
            nc.vector.memset(pad2[:, :, 0:1], 0.0)
            nc.vector.memset(pad2[:, :, W + 1:W + 2], 0.0)
            pad1 = big.tile([C, SROWS, WP], F16, tag="pad1")
            nc.vector.memset(pad1[:, :, 0:1], 0.0)
            nc.vector.memset(pad1[:, :, W + 1:W + 2], 0.0)

            # per image: ss -> sqrt (PSUM->SBUF evac) -> 128-wide reciprocal
            # -> broadcast + scale into the padded interiors
            def norm_img(sq, base, npix, rawp, pad, prow0, nTw, chunks):
                for (j, o, n) in chunks:
                    ssp = nps.tile([1, 512], F32, tag="nps", name="ssp")
                    nc.tensor.matmul(
                        ssp[:, :n], ones128[32 * j:32 * j + 32, :],
                        sq[32 * j:32 * j + 32, o:o + n],
                        start=True, stop=True, tile_position=(32 * j, 0))
                    pos = base + chunks_pos(j, o)
                    nc.scalar.activation(
                        nrow[:, pos:pos + n], ssp[:, :n],
                        mybir.ActivationFunctionType.Sqrt,
                        bias=eps2c[0:1, :])
                nT = work.tile([128, npix // 128], F32, tag=nTw, name=nTw)
                nc.sync.dma_start(
                    out=nT, in_=nrow[:, base:base + npix].rearrange(
                        "a (p c) -> a p c", p=128))
                rT = work.tile([128, npix // 128], F32R, tag=nTw + "r",
                               name=nTw + "r")
                with nc.allow_low_precision(reason="f32r 1/norm, 12 bits"):
                    nc.vector.reciprocal(rT, nT)
                nc.sync.dma_start(
                    out=rrow[:, base:base + npix].rearrange(
                        "a (p c) -> a p c", p=128), in_=rT)
                for (j, o, n) in chunks:
                    rb = nps.tile([C, 512], F32, tag="nps", name="rb")
                    g = chunks_pos(j, o)
                    nc.tensor.matmul(rb[:, :n], onesb,
                                     rrow[:, base + g:base + g + n],
                                     start=True, stop=True)
                    r0 = prow0 + g // W
                    nc.vector.tensor_mul(
                        pad[:, r0:r0 + n // W, 1:W + 1],
                        rawp[32 * j:32 * j + 32, o:o + n],
                        rb[:, :n],
                    )

            chunks2 = [(j, o, 512) for j in range(4) for o in (0, 512)]
            chunks_pos = lambda j, o: XS2 * j + o  # noqa: E731
            norm_img(sq2, 0, L, raw2, pad2, 1, "nT2", chunks2)
            chunks1 = []
            for j in range(4):
                chunks1.append((j, 0, 512))
                if PX1[j] > 512:
                    chunks1.append((j, 512, PX1[j] - 512))
            chunks_pos = lambda j, o: RST1[j] * W + o  # noqa: E731
            norm_img(sq1, L, n1, raw1, pad1, 0, "nT1", chunks1)

            # ---- contiguous d-major y-tap patch tensors over the PADDED
            # geometry.  kc[32j+c, 1+f] = pad2[c, j + f//66, f%66] so the
            # dx-shifted stationary for l-tile t is the flat single-free-dim
            # window kc[:, 128t+dx : 128t+dx+128].  One contiguous DMA per
            # tap.  Guard cells at both ends cover the t=0/dx=0 underrun. ----
            kc = big.tile([96, LPAD + 2], F16, tag="kc")
            nc.gpsimd.memset(kc[:, 0:1], 0.0)
            nc.gpsimd.memset(kc[:, LPAD + 1:LPAD + 2], 0.0)
            qc = big.tile([96, QROWS, WP], F16, tag="qc")
            pad2f = pad2.rearrange("c h w -> c (h w)")
            pad1f = pad1.rearrange("c h w -> c (h w)")
            qcf = qc.rearrange("c h w -> c (h w)")
            dma_engs = [nc.sync, nc.scalar, nc.gpsimd]
            for j in range(3):
                dma_engs[j].dma_start(
                    out=kc[32 * j:32 * j + 32, 1:LPAD + 1],
                    in_=pad2f[:, WP * j:WP * j + LPAD])
                dma_engs[j].dma_start(
                    out=qcf[32 * j:32 * j + 32, :],
                    in_=pad1f[:, WP * j:WP * j + QROWS * WP])

            # ---- main loop: 3 accumulated score matmuls -> exp -> stats ----
            n_rounds = N_QT * N_LT
            stats_t = [None] * N_QT
            pt = []
            for i in range(NP):
                pt.append(pp.tile([128, 512], BF16, tag=f"p_{i}",
                                  name=f"p_{i}"))

            stats_sb = epi.tile([3, NQ], F32, tag="stats_sb", bufs=1)

            def stats_mm(r):
                qt, t = divmod(r, N_LT)
                if t == 0:
                    stats_t[qt] = stps.tile([3, 512], F32, tag="stats",
                                            name="stats")
                nc.tensor.matmul(stats_t[qt], w3r[:, 3 * t:3 * t + 3],
                                 pt[r % NP],
                                 start=(t == 0), stop=(t == N_LT - 1))
                if t == N_LT - 1:
                    nc.scalar.copy(stats_sb[:, 512 * qt:512 * (qt + 1)],
                                   stats_t[qt])

            done = 0
            for r in range(n_rounds):
                qt, t = divmod(r, N_LT)
                psS = sps.tile([128, 512], F32, tag="S", name="psS")
                for dx in range(3):
                    nc.tensor.matmul(
                        psS, kc[:, 128 * t + dx:128 * t + dx + 128],
                        qc[:, 8 * qt:8 * qt + 8, dx:dx + W],
                        start=(dx == 0), stop=(dx == 2))
                nc.scalar.activation(pt[r % NP], psS,
                                     mybir.ActivationFunctionType.Exp,
                                     bias=shiftc, scale=SCALE)
                # bundle stats 4 per group: the PE pays its stationary
                # shape-switch cost twice per 4 rounds instead of twice/round
                if r % 4 == 3 and r >= 7:
                    for rr in range(r - 7, r - 3):
                        stats_mm(rr)
                        done = rr + 1
            for rr in range(done, n_rounds):
                stats_mm(rr)

            # ---- epilogue: flow = S/Z - coord ----
            zT = epi.tile([128, NQT], F32, tag="zT")
            nc.sync.dma_start(out=zT,
                              in_=stats_sb[0:1, :].rearrange("a (p c) -> a p c", p=128))
            syT = epi.tile([128, NQT], F32, tag="syT")
            nc.sync.dma_start(out=syT,
                              in_=stats_sb[1:2, :].rearrange("a (p c) -> a p c", p=128))
            sxT = epi.tile([128, NQT], F32, tag="sxT")
            nc.sync.dma_start(out=sxT,
                              in_=stats_sb[2:3, :].rearrange("a (p c) -> a p c", p=128))
            rz = epi.tile([128, NQT], F32R, tag="rz")
            with nc.allow_low_precision(reason="f32r 1/Z, 12 bits"):
                nc.vector.reciprocal(rz, zT)
            fw = epi.tile([128, NQT], F32, tag="fw")
            nc.vector.tensor_mul(fw, sxT, rz)
            nc.vector.tensor_sub(fw, fw, xqs)
            fh = epi.tile([128, NQT], F32, tag="fh")
            nc.vector.tensor_mul(fh, syT, rz)
            nc.vector.tensor_sub(fh, fh, yqs)
            nc.sync.dma_start(
                out=outp[0:1, :].rearrange("a (p c) -> a p c", p=128), in_=fw)
            nc.sync.dma_start(
                out=outp[1:2, :].rearrange("a (p c) -> a p c", p=128), in_=fh)

    nc.finalize()
    return nc


def _host_consts():
    p = np.arange(128)
    w3 = np.zeros((128, 3 * N_LT), np.float32)
    for t in range(N_LT):
        f = 128 * t + p                    # padded flat position
        xx = f % WP
        yy = f // WP
        valid = ((xx >= 1) & (xx <= W) & (yy < H)).astype(np.float32)
        w3[:, 3 * t] = valid
        w3[:, 3 * t + 1] = yy * valid      # global iy
        w3[:, 3 * t + 2] = (xx - 1) * valid  # global ix
    q = np.arange(NQ).reshape(128, NQT)    # [128, NQT], q = NQT*p + c
    xq = (q % W).astype(np.float32)
    ly = (q // W).astype(np.float32)
    return w3, xq, ly


def kernel(feature1, feature2):
    feature1 = np.ascontiguousarray(feature1, np.float32)
    feature2 = np.ascontiguousarray(feature2, np.float32)
    w3, xq, ly = _host_consts()

    f1p = np.zeros((B, C, H + 2, W), np.float32)
    f1p[:, :, 1:H + 1, :] = feature1

    in_maps = []
    for core in range(N_CORES):
        b, h = divmod(core, 2)
        in_maps.append({
            "f1h": np.ascontiguousarray(f1p[b, :, h * QROWS:h * QROWS + SROWS, :]),
            "f2": np.ascontiguousarray(feature2[b]),
            "w3": w3,
            "yqt": ly + h * QROWS,
            "xqt": xq,
        })

    if "nc" not in _NC_CACHE:
        _NC_CACHE["nc"] = _build_nc()
    res = bass_utils.run_bass_kernel_spmd(
        _NC_CACHE["nc"], in_maps, core_ids=list(range(N_CORES)))
    global _LAST_RES
    _LAST_RES = res

    out = np.zeros((B, 2, H, W), np.float32)
    for core in range(N_CORES):
        b, h = divmod(core, 2)
        out[b, :, h * QROWS:(h + 1) * QROWS, :] = (
            res.results[core]["outp"].reshape(2, QROWS, W))
    return out


# revision 14
# speedup vs baseline: 1.3294x; 1.0005x over previous
"""Trainium2 Bass kernel for the correlation-softargmax flow module.

Math (per batch b, query pixel q=(y,x)):
  c1 = l2norm_C(feature1), warp = l2norm_C(feature2)
  s[l,q] = <3x3 patch of warp at l, 3x3 patch of c1 at q>    (D = 32*9 = 288)
  p = softmax_l(10*s);  flow = (E_p[ix_l] - x, E_p[iy_l] - y)

Design ("3mm"): the 9 patch taps factor as (3 y-taps) x (3 x-shifts).  The
y-taps fold into the matmul contraction (K = 32ch * 3 = 96).  The x-shift sum
is done PRE-exp by accumulating three matmuls into one PSUM bank.  One Exp
(scale 10, bias -30 keeps everything finite: |10 s| <= 90) yields p directly;
a [128,3] stats matmul accumulates Z, Sy, Sx over l per query (flash-style).

The l axis is tiled over the X-PADDED image geometry: kc is ONE contiguous
[96, 66*64] tensor (plus guard cells) holding the 3 y-taps of the x-padded
normalized f2; the dx-shifted stationary for l-tile t is the flat window
kc[:, 128t+dx : 128t+dx+128] - a single-free-dim AP as the PE requires, with
no per-dx tensor copies.  Out-of-image taps read the zero pad columns; the
~3% of partition slots that land ON a pad column are junk l-positions whose
stats weights are zero, so they never contribute.  33 l-tiles cover all 4224
padded positions.  Stats matmuls are bundled 4-per-group so the PE pays the
(~100 ns) stationary shape-switch penalty twice per 4 rounds instead of
twice per round.

Per round: PE does 4x512 matmul columns, Act one exp - no DVE multiplies,
no shift DMAs, so the PE stays continuously busy at its full 2.4 GHz p-state.

Sharding: 8 cores = 4 batches x 2 query-row halves. Each core holds the full
K-side image (softmax runs over all 4096 l) and 2048 queries.
"""

import sys

import numpy as np

sys.path.insert(0, "/opt/trn_rl_repo")

import concourse.bass as bass  # noqa: E402
import concourse.mybir as mybir  # noqa: E402
import concourse.tile as tile  # noqa: E402
from concourse import bacc, bass_utils  # noqa: E402

F32 = mybir.dt.float32
F32R = mybir.dt.float32r
F16 = mybir.dt.float16
BF16 = mybir.dt.bfloat16

B, C, H, W = 4, 32, 64, 64
L = H * W              # 4096 match locations
NQ = L // 2            # queries per core
QROWS = H // 2         # query rows per core
SROWS = QROWS + 2      # f1 halo slab rows
NQT = NQ // 128        # epilogue bounce columns
N_CORES = 8
SCALE = 10.0
SHIFT = -30.0          # fixed exp shift: p = exp(10*s - 30), |10 s| <= 90
EPS = 1e-12
WP = W + 2             # x-padded row width (66)
LPAD = WP * H          # 4224 padded l positions
N_LT = LPAD // 128     # 33 l-tiles
N_QT = NQ // 512       # 4 q-tiles

_NC_CACHE = {}
_LAST_RES = None


def _build_nc():
    nc = bacc.Bacc(None, target_bir_lowering=False)

    f1h = nc.dram_tensor("f1h", [C, SROWS, W], F32, kind="ExternalInput")
    f2 = nc.dram_tensor("f2", [C, H, W], F32, kind="ExternalInput")
    w3 = nc.dram_tensor("w3", [128, 3 * N_LT], F32, kind="ExternalInput")
    yqt = nc.dram_tensor("yqt", [128, NQT], F32, kind="ExternalInput")
    xqt = nc.dram_tensor("xqt", [128, NQT], F32, kind="ExternalInput")
    outp = nc.dram_tensor("outp", [2, NQ], F32, kind="ExternalOutput")

    n1 = SROWS * W         # 2176 pixels in the f1 halo slab
    NP = 10                # p ring depth
    # f1 norm blocks must be row-aligned for padded-image writes: rows
    # [9, 9, 8, 8] -> pixel counts [576, 576, 512, 512]
    ROWS1 = [9, 9, 8, 8]
    RST1 = [0, 9, 18, 26]
    PX1 = [r * W for r in ROWS1]

    with tile.TileContext(nc) as tc:
        with tc.tile_pool(name="big", bufs=1) as big, \
             tc.tile_pool(name="work", bufs=1) as work, \
             tc.tile_pool(name="small", bufs=1) as small, \
             tc.tile_pool(name="pp", bufs=1) as pp, \
             tc.tile_pool(name="epi", bufs=1) as epi, \
             tc.tile_pool(name="nps", bufs=2, space="PSUM") as nps, \
             tc.tile_pool(name="sps", bufs=4, space="PSUM") as sps, \
             tc.tile_pool(name="stps", bufs=2, space="PSUM") as stps:

            # ---- load inputs (images packed 4 channel-blocks deep so the
            # squares run 128 partitions wide; f1 blocks are row-aligned) ----
            XS2 = L // 4           # 1024 pixels per f2 block
            XS1 = max(PX1)         # 576-wide f1 block staging
            raw2 = big.tile([128, XS2], F32, tag="raw2")
            f2f = f2.rearrange("c h w -> c (h w)")
            raw1 = big.tile([128, XS1], F32, tag="raw1")
            f1f = f1h.rearrange("c h w -> c (h w)")
            for j in range(4):
                nc.sync.dma_start(out=raw2[32 * j:32 * j + 32, :],
                                  in_=f2f[:, XS2 * j:XS2 * (j + 1)])
                s0 = RST1[j] * W
                nc.gpsimd.dma_start(out=raw1[32 * j:32 * j + 32, 0:PX1[j]],
                                    in_=f1f[:, s0:s0 + PX1[j]])
            w3f = small.tile([128, 3 * N_LT], F32, tag="w3f")
            nc.scalar.dma_start(out=w3f, in_=w3[:, :])
            xqs = small.tile([128, NQT], F32, tag="xqs")
            nc.scalar.dma_start(out=xqs, in_=xqt[:, :])
            yqs = small.tile([128, NQT], F32, tag="yqs")
            nc.scalar.dma_start(out=yqs, in_=yqt[:, :])

            onesf = small.tile([128, 1], F32, tag="onesf")
            nc.vector.memset(onesf, 1.0)
            ones128 = small.tile([128, 1], F32R, tag="ones128")
            nc.scalar.copy(ones128, onesf)
            shiftc = small.tile([128, 1], F32, tag="shiftc")
            nc.vector.memset(shiftc, SHIFT)
            eps2c = small.tile([128, 1], F32, tag="eps2c")
            nc.vector.memset(eps2c, EPS * EPS)

            # ---- l2 normalization over C ----
            sq2 = big.tile([128, XS2], F32R, tag="sq2")
            nc.vector.tensor_mul(sq2, raw2, raw2)
            sq1 = big.tile([128, XS1], F32R, tag="sq1")
            nc.vector.tensor_mul(sq1, raw1, raw1)

            w3r = small.tile([128, 3 * N_LT], BF16, tag="w3r")
            nc.scalar.copy(w3r, w3f)
            onesbf = small.tile([1, C], F32, tag="onesbf")
            nc.vector.memset(onesbf, 1.0)
            onesb = small.tile([1, C], F32R, tag="onesb")
            nc.scalar.copy(onesb, onesbf)

            nrow = work.tile([1, L + n1], F32, tag="nrow")
            rrow = work.tile([1, L + n1], F32R, tag="rrow")

            # x-AND-y padded fp16 images; interiors written by the norm
            # scale multiplies through row-aligned strided views
            pad2 = big.tile([C, H + 2, WP], F16, tag="pad2")
            nc.vector.memset(pad2[:, 0:1, :], 0.0)
            nc.vector.memset(pad2[:, H + 1:H + 2, :], 0.0)
            nc.vector.memset(pad2[:, :, 0:1], 0.0)
            nc.vector.memset(pad2[:, :, W + 1:W + 2], 0.0)
            pad1 = big.tile([C, SROWS, WP], F16, tag="pad1")
            nc.vector.memset(pad1[:, :, 0:1], 0.0)
            nc.vector.memset(pad1[:, :, W + 1:W + 2], 0.0)

            # per image: ss -> sqrt (PSUM->SBUF evac) -> 128-wide reciprocal
            # -> broadcast + scale into the padded interiors
            def norm_img(sq, base, npix, rawp, pad, prow0, nTw, chunks):
                for (j, o, n) in chunks:
                    ssp = nps.tile([1, 512], F32, tag="nps", name="ssp")
                    nc.tensor.matmul(
                        ssp[:, :n], ones128[32 * j:32 * j + 32, :],
                        sq[32 * j:32 * j + 32, o:o + n],
                        start=True, stop=True, tile_position=(32 * j, 0))
                    pos = base + chunks_pos(j, o)
                    nc.scalar.activation(
                        nrow[:, pos:pos + n], ssp[:, :n],
                        mybir.ActivationFunctionType.Sqrt,
                        bias=eps2c[0:1, :])
                nT = work.tile([128, npix // 128], F32, tag=nTw, name=nTw)
                nc.sync.dma_start(
                    out=nT, in_=nrow[:, base:base + npix].rearrange(
                        "a (p c) -> a p c", p=128))
                rT = work.tile([128, npix // 128], F32R, tag=nTw + "r",
                               name=nTw + "r")
                with nc.allow_low_precision(reason="f32r 1/norm, 12 bits"):
                    nc.vector.reciprocal(rT, nT)
                nc.sync.dma_start(
                    out=rrow[:, base:base + npix].rearrange(
                        "a (p c) -> a p c", p=128), in_=rT)
                for (j, o, n) in chunks:
                    rb = nps.tile([C, 512], F32, tag="nps", name="rb")
                    g = chunks_pos(j, o)
                    nc.tensor.matmul(rb[:, :n], onesb,
                                     rrow[:, base + g:base + g + n],
                                     start=True, stop=True)
                    r0 = prow0 + g // W
                    nc.vector.tensor_mul(
                        pad[:, r0:r0 + n // W, 1:W + 1],
                        rawp[32 * j:32 * j + 32, o:o + n],
                        rb[:, :n],
                    )

            chunks2 = [(j, o, 512) for j in range(4) for o in (0, 512)]
            chunks_pos = lambda j, o: XS2 * j + o  # noqa: E731
            norm_img(sq2, 0, L, raw2, pad2, 1, "nT2", chunks2)
            chunks1 = []
            for j in range(4):
                chunks1.append((j, 0, 512))
                if PX1[j] > 512:
                    chunks1.append((j, 512, PX1[j] - 512))
            chunks_pos = lambda j, o: RST1[j] * W + o  # noqa: E731
            norm_img(sq1, L, n1, raw1, pad1, 0, "nT1", chunks1)

            # ---- contiguous d-major y-tap patch tensors over the PADDED
            # geometry.  kc[32j+c, 1+f] = pad2[c, j + f//66, f%66] so the
            # dx-shifted stationary for l-tile t is the flat single-free-dim
            # window kc[:, 128t+dx : 128t+dx+128].  One contiguous DMA per
            # tap.  Guard cells at both ends cover the t=0/dx=0 underrun. ----
            kc = big.tile([96, LPAD + 2], F16, tag="kc")
            nc.gpsimd.memset(kc[:, 0:1], 0.0)
            nc.gpsimd.memset(kc[:, LPAD + 1:LPAD + 2], 0.0)
            qc = big.tile([96, QROWS, WP], F16, tag="qc")
            pad2f = pad2.rearrange("c h w -> c (h w)")
            pad1f = pad1.rearrange("c h w -> c (h w)")
            qcf = qc.rearrange("c h w -> c (h w)")
            dma_engs = [nc.sync, nc.scalar, nc.gpsimd]
            for j in range(3):
                dma_engs[j].dma_start(
                    out=kc[32 * j:32 * j + 32, 1:LPAD + 1],
                    in_=pad2f[:, WP * j:WP * j + LPAD])
                dma_engs[j].dma_start(
                    out=qcf[32 * j:32 * j + 32, :],
                    in_=pad1f[:, WP * j:WP * j + QROWS * WP])

            # ---- main loop: 3 accumulated score matmuls -> exp -> stats ----
            n_rounds = N_QT * N_LT
            stats_t = [None] * N_QT
            pt = []
            for i in range(NP):
                pt.append(pp.tile([128, 512], BF16, tag=f"p_{i}",
                                  name=f"p_{i}"))

            stats_sb = epi.tile([3, NQ], F32, tag="stats_sb", bufs=1)

            def stats_mm(r):
                qt, t = divmod(r, N_LT)
                if t == 0:
                    stats_t[qt] = stps.tile([3, 512], F32, tag="stats",
                                            name="stats")
                nc.tensor.matmul(stats_t[qt], w3r[:, 3 * t:3 * t + 3],
                                 pt[r % NP],
                                 start=(t == 0), stop=(t == N_LT - 1))
                if t == N_LT - 1:
                    nc.scalar.copy(stats_sb[:, 512 * qt:512 * (qt + 1)],
                                   stats_t[qt])

            done = 0
            for r in range(n_rounds):
                qt, t = divmod(r, N_LT)
                psS = sps.tile([128, 512], F32, tag="S", name="psS")
                for dx in range(3):
                    nc.tensor.matmul(
                        psS, kc[:, 128 * t + dx:128 * t + dx + 128],
                        qc[:, 8 * qt:8 * qt + 8, dx:dx + W],
                        start=(dx == 0), stop=(dx == 2))
                nc.scalar.activation(pt[r % NP], psS,
                                     mybir.ActivationFunctionType.Exp,
                                     bias=shiftc, scale=SCALE)
                if r % 4 == 3 and r >= 7:
                    for rr in range(r - 7, r - 3):
                        stats_mm(rr)
                        done = rr + 1
            for rr in range(done, n_rounds):
                stats_mm(rr)

            # ---- epilogue: flow = S/Z - coord ----
            zT = epi.tile([128, NQT], F32, tag="zT")
            nc.sync.dma_start(out=zT,
                              in_=stats_sb[0:1, :].rearrange("a (p c) -> a p c", p=128))
            syT = epi.tile([128, NQT], F32, tag="syT")
            nc.sync.dma_start(out=syT,
                              in_=stats_sb[1:2, :].rearrange("a (p c) -> a p c", p=128))
            sxT = epi.tile([128, NQT], F32, tag="sxT")
            nc.sync.dma_start(out=sxT,
                              in_=stats_sb[2:3, :].rearrange("a (p c) -> a p c", p=128))
            rz = epi.tile([128, NQT], F32R, tag="rz")
            with nc.allow_low_precision(reason="f32r 1/Z, 12 bits"):
                nc.vector.reciprocal(rz, zT)
            fw = epi.tile([128, NQT], F32, tag="fw")
            nc.vector.tensor_mul(fw, sxT, rz)
            nc.vector.tensor_sub(fw, fw, xqs)
            fh = epi.tile([128, NQT], F32, tag="fh")
            nc.vector.tensor_mul(fh, syT, rz)
            nc.vector.tensor_sub(fh, fh, yqs)
            nc.sync.dma_start(
                out=outp[0:1, :].rearrange("a (p c) -> a p c", p=128), in_=fw)
            nc.sync.dma_start(
                out=outp[1:2, :].rearrange("a (p c) -> a p c", p=128), in_=fh)

    nc.finalize()
    return nc


def _host_consts():
    p = np.arange(128)
    w3 = np.zeros((128, 3 * N_LT), np.float32)
    for t in range(N_LT):
        f = 128 * t + p                    # padded flat position
        xx = f % WP
        yy = f // WP
        valid = ((xx >= 1) & (xx <= W) & (yy < H)).astype(np.float32)
        w3[:, 3 * t] = valid
        w3[:, 3 * t + 1] = yy * valid      # global iy
        w3[:, 3 * t + 2] = (xx - 1) * valid  # global ix
    q = np.arange(NQ).reshape(128, NQT)    # [128, NQT], q = NQT*p + c
    xq = (q % W).astype(np.float32)
    ly = (q // W).astype(np.float32)
    return w3, xq, ly


def kernel(feature1, feature2):
    feature1 = np.ascontiguousarray(feature1, np.float32)
    feature2 = np.ascontiguousarray(feature2, np.float32)
    w3, xq, ly = _host_consts()

    f1p = np.zeros((B, C, H + 2, W), np.float32)
    f1p[:, :, 1:H + 1, :] = feature1

    in_maps = []
    for core in range(N_CORES):
        b, h = divmod(core, 2)
        in_maps.append({
            "f1h": np.ascontiguousarray(f1p[b, :, h * QROWS:h * QROWS + SROWS, :]),
            "f2": np.ascontiguousarray(feature2[b]),
            "w3": w3,
            "yqt": ly + h * QROWS,
            "xqt": xq,
        })

    if "nc" not in _NC_CACHE:
        _NC_CACHE["nc"] = _build_nc()
    res = bass_utils.run_bass_kernel_spmd(
        _NC_CACHE["nc"], in_maps, core_ids=list(range(N_CORES)))
    global _LAST_RES
    _LAST_RES = res

    out = np.zeros((B, 2, H, W), np.float32)
    for core in range(N_CORES):
        b, h = divmod(core, 2)
        out[b, :, h * QROWS:(h + 1) * QROWS, :] = (
            res.results[core]["outp"].reshape(2, QROWS, W))
    return out
